# revision 22
# baseline (speedup 1.0000x reference)
"""Two-layer GCN (PyG GCNConv-style) on 8 Trainium2 NeuronCores.

Strategy: nodes are partitioned across the 8 cores (load-balanced into
128-row tiles by in-degree), edges partitioned by destination node so
the segment-sum is local.  Each layer is transform-first: local GEMM
(h = x @ W, scaled by dinv[src]), AllGather of the bf16 transformed
features, then a local gather + segment-sum over incoming edges.

Key optimizations over the v1 kernel (1.04 ms):
- S (the one-hot segment-sum matrix) is fp8 {0,1}: the dinv[dst]
  normalization is factored out of S — dinv[src] is folded into the
  AllGathered features (as before), dinv[dst] is applied after the
  aggregation.  For layer 1, relu(dinv*x + b1) = dinv*relu(x + b1*rdinv)
  lets the dinv[dst] scale commute past the relu and fold into the
  layer-2 GEMM output scale (dinv^2 instead of dinv); the bias enters
  through the PSUM-init outer-product matmul with rdinv = 1/dinv.
  This halves the dense-S DMA traffic and keeps everything exact.
- Gathers are batched: one dma_gather per (batch of 4 dst tiles,
  int16 window) instead of per tile, cutting the per-instruction SWDGE
  ucode overhead (~1 us each) 6.5x.  All other DMAs (x-tiles, S tiles,
  self rows, outputs) are batched the same way to cut the ~600 ns
  per-dma_start sequencer cost.
- The AllGathers are chunked (tiles [0,16), [16,32), [32,49)) so the
  collective overlaps the GEMM (layer 1) / aggregation (layer 2) that
  produces its input.  The global feature table is laid out
  chunk-major, then core-major, then partition-major so each chunk is
  a contiguous AllGather destination; the gather indices address this
  layout directly.

The segment-sum runs on the TensorEngine: for each destination tile of
128 nodes, its incoming edges (chunked by 128) are bulk-gathered with
dma_gather into SBUF [128edges x F] per chunk (bf16), and contracted
with the one-hot S.  Layer-1 aggregation is computed TRANSPOSED
(psxT[f,d] = sum_k G_k^T @ S_k) so the relu output lands directly in
lhsT layout for the layer-2 GEMM - no on-device transposes at all.

dma_gather takes int16 row indices, so the gathered table is addressed
through two overlapping <=32767-row windows.
"""

import numpy as np

P = 128
N_CORES = 8
WINDOW_CAP = 32512  # dma_gather int16 window (multiple of 128, <= 32767)
BT = 2  # dst tiles per gather/DMA batch

_prog_cache = {}


# ---------------------------------------------------------------- host side


def _bf16(a):
    import ml_dtypes

    return np.asarray(a, dtype=ml_dtypes.bfloat16)


def _fp8(a):
    import ml_dtypes

    return np.asarray(a, dtype=ml_dtypes.float8_e4m3)


def _ag_chunks(T, B):
    """One AllGather per layer (a Shared tensor allows only a single
    collective writer, and splitting the gather table into per-chunk
    windows inflates the per-tile chunk count ~12%)."""
    return [(0, T)]


def _batches(t0, t1, B):
    return [(s, min(s + B, t1) - s) for s in range(t0, t1, B)]


def _preprocess(x, edge_index):
    """Partition nodes/edges, build per-core device arrays."""
    x = np.ascontiguousarray(np.asarray(x, dtype=np.float32))
    ei = np.asarray(edge_index)
    N, IN = x.shape

    src = ei[0].astype(np.int64)
    dst = ei[1].astype(np.int64)

    deg = 1 + np.bincount(dst, minlength=N)  # with self loop, >= 1
    dinv = (1.0 / np.sqrt(deg.astype(np.float64))).astype(np.float32)
    rdinv_node = np.sqrt(deg.astype(np.float64)).astype(np.float32)

    npc_nodes = -(-N // N_CORES)
    T = -(-npc_nodes // P)  # dst tiles per core
    NPC = T * P  # node slots per core
    n_tiles = N_CORES * T
    NG = n_tiles * P  # global node slots

    # --- pack nodes into tiles, balancing per-tile in-degree (LPT) ----
    import heapq

    degg = deg - 1  # gathered (non-self) in-degree
    tile_of = np.empty(N, dtype=np.int64)
    pos_of = np.empty(N, dtype=np.int64)
    counts = np.zeros(n_tiles, dtype=np.int64)
    loads = np.zeros(n_tiles, dtype=np.int64)
    order = np.argsort(-degg, kind="stable")
    heap = [(0, t) for t in range(n_tiles)]
    heapq.heapify(heap)
    deg_l = degg[order]
    for i in range(N):
        v = order[i]
        while True:
            load, t = heapq.heappop(heap)
            if counts[t] < P:
                break
        tile_of[v] = t
        pos_of[v] = counts[t]
        counts[t] += 1
        load += int(deg_l[i])
        loads[t] = load
        if counts[t] < P:
            heapq.heappush(heap, (load, t))

    # repair pass: move small nodes off overloaded tiles to reach the
    # ideal chunk count ceil(total/(n_tiles*P)) if possible
    K_ideal = max(1, int(-(-int(degg.sum()) // (n_tiles * P))))
    target = K_ideal * P
    if loads.max() > target:
        by_tile = [[] for _ in range(n_tiles)]
        for i in range(N - 1, -1, -1):  # ascending degree order
            by_tile[tile_of[order[i]]].append(order[i])
        free = [(loads[t], t) for t in range(n_tiles)
                if counts[t] < P and loads[t] < target]
        heapq.heapify(free)
        for t_over in np.flatnonzero(loads > target):
            stack = by_tile[t_over]
            si = 0
            while loads[t_over] > target and si < len(stack) and free:
                v = stack[si]
                si += 1
                d = int(degg[v])
                moved = False
                tried = []
                while free:
                    lo, t2 = heapq.heappop(free)
                    if lo != loads[t2] or counts[t2] >= P:
                        continue  # stale
                    if loads[t2] + d <= target:
                        tile_of[v] = t2
                        pos_of[v] = counts[t2]
                        counts[t2] += 1
                        loads[t2] += d
                        loads[t_over] -= d
                        moved = True
                        if counts[t2] < P and loads[t2] < target:
                            heapq.heappush(free, (loads[t2], t2))
                        break
                    tried.append((lo, t2))
                for it in tried:
                    heapq.heappush(free, it)
                if not moved:
                    break
        # recompute pos_of consistently (holes possible after moves)
        ordv = np.lexsort((np.arange(N), tile_of))
        pos = np.empty(N, dtype=np.int64)
        tt = tile_of[ordv]
        st = np.zeros(n_tiles + 1, dtype=np.int64)
        np.cumsum(np.bincount(tt, minlength=n_tiles), out=st[1:])
        pos[ordv] = np.arange(N) - st[tt]
        pos_of = pos

    K = max(1, int(-(-loads.max() // P)))  # min gather chunks per dst tile

    # --- global table row of each node: chunk-major, core-major, then
    # partition-major / tile-minor within the chunk (matches the
    # p-major flattening of the per-core [P, qlen*F] AllGather input).
    agch = _ag_chunks(T, BT)
    chunk_of_t = np.empty(T, dtype=np.int64)
    chunk_t0 = np.empty(T, dtype=np.int64)
    chunk_len = np.empty(T, dtype=np.int64)
    rowbase_t = np.empty(T, dtype=np.int64)
    base = 0
    for qi, (tq0, tq1) in enumerate(agch):
        qlen = tq1 - tq0
        chunk_of_t[tq0:tq1] = qi
        chunk_t0[tq0:tq1] = tq0
        chunk_len[tq0:tq1] = qlen
        rowbase_t[tq0:tq1] = base
        base += N_CORES * P * qlen

    n_core = (tile_of // T).astype(np.int64)
    n_t = tile_of % T
    # row(q, c, p, t) = rowbase_q + c*P*qlen + p*qlen + (t - tq0)
    row_of = (
        rowbase_t[n_t]
        + n_core * P * chunk_len[n_t]
        + pos_of * chunk_len[n_t]
        + (n_t - chunk_t0[n_t])
    )

    # --- per-edge placement (non-self edges) --------------------------
    e_tile = tile_of[dst]
    e_dslot = pos_of[dst].astype(np.int64)
    e_srcrow = row_of[src]

    sort_idx = np.lexsort((e_srcrow, e_tile))
    e_tile = e_tile[sort_idx]
    e_dslot = e_dslot[sort_idx]
    e_srcrow = e_srcrow[sort_idx]
    nE = len(e_tile)

    # --- window split (dma_gather int16 limit) ------------------------
    WA = min(WINDOW_CAP, NG)  # window A = rows [0, WA)
    WB_off = max(NG - WINDOW_CAP, 0)  # window B = rows [WB_off, NG)
    use_B = WB_off > 0

    tile_n = np.bincount(e_tile, minlength=n_tiles)
    if use_B:
        mustA = e_srcrow < WB_off
        mustB = e_srcrow >= WA
        flex = ~mustA & ~mustB
        cntA = np.bincount(e_tile[mustA], minlength=n_tiles)
        cntB = np.bincount(e_tile[mustB], minlength=n_tiles)
        # find (K_A, K_B) with K_A+K_B minimal and all tiles feasible
        found = None
        K_tot = K
        while found is None:
            mid = -(-K_tot // 2)
            for d in range(K_tot + 1):
                for K_A in {mid + d, mid - d}:
                    if not 0 <= K_A <= K_tot:
                        continue
                    K_B = K_tot - K_A
                    if (
                        cntA.max() <= K_A * P
                        and cntB.max() <= K_B * P
                        and tile_n.max() <= (K_A + K_B) * P
                    ):
                        found = (K_A, K_B)
                        break
                if found:
                    break
            if not found:
                K_tot += 1
        K_A, K_B = found
        capB = K_B * P
        # how many of each tile's flex edges go to window A
        nA_t = np.minimum(K_A * P, cntA + np.bincount(
            e_tile[flex], minlength=n_tiles))
        nA_t = np.maximum(nA_t, tile_n - capB)
        flexA_quota = nA_t - cntA
        flex_idx = np.flatnonzero(flex)
        ft = e_tile[flex_idx]
        fstart = np.zeros(n_tiles + 1, dtype=np.int64)
        np.cumsum(np.bincount(ft, minlength=n_tiles), out=fstart[1:])
        frank = np.arange(len(ft)) - fstart[ft]
        toA = mustA.copy()
        toA[flex_idx[frank < flexA_quota[ft]]] = True
    else:
        K_A, K_B = K, 0
        toA = np.ones(nE, dtype=bool)
    K_tot = K_A + K_B
    KC = K_tot + 1  # chunk columns per tile incl. the self chunk

    # --- chunk/slot assignment within each (tile, window) -------------
    e_j = np.empty(nE, dtype=np.int64)  # slot within its window list
    e_idxval = np.empty(nE, dtype=np.int64)  # int16 index value
    for is_A in (True, False):
        m = toA if is_A else ~toA
        if not m.any():
            continue
        idxs = np.flatnonzero(m)
        t_sel = e_tile[idxs]
        start = np.zeros(n_tiles + 1, dtype=np.int64)
        np.cumsum(np.bincount(t_sel, minlength=n_tiles), out=start[1:])
        e_j[idxs] = np.arange(len(idxs)) - start[t_sel]
        e_idxval[idxs] = e_srcrow[idxs] - (0 if is_A else WB_off)

    e_kloc = e_j // P  # chunk within window
    e_p = e_j % P
    e_chunk = np.where(toA, e_kloc, K_A + e_kloc)  # chunk within tile

    e_core = e_tile // T
    e_t_in_core = e_tile % T

    # idx table: window-A blocks first (tile t at cols [t*K_A*8, ...)),
    # then window-B blocks at T*K_A*8 + t*K_B*8.  Value for position j
    # of a tile's window list sits at [j%16, base + j//16], replicated
    # across the 8 groups of 16 partitions.
    idx_cols = T * K_tot * 8
    idx16 = np.zeros((N_CORES, 16, max(idx_cols, 8)), dtype=np.int16)
    blk_base = np.where(
        toA,
        e_t_in_core * K_A * 8,
        T * K_A * 8 + e_t_in_core * K_B * 8,
    )
    idx16[e_core, e_j % 16, blk_base + e_j // 16] = e_idxval.astype(np.int16)
    idxT = np.tile(idx16, (1, P // 16, 1))

    # dense one-hot S (fp8 on the wire): S[p, col*P + d] = 1 for each edge
    S = np.zeros((N_CORES, P, T * KC * P), dtype=np.float32)
    col = e_t_in_core * KC + e_chunk
    S[e_core, e_p, col * P + e_dslot] = 1.0

    # self chunk (k == K_tot): identity
    n_slot = pos_of
    scol = n_t * KC + K_tot
    S[n_core, n_slot, scol * P + n_slot] = 1.0

    # per-node scale vectors; 0 for empty slots
    dinvn = np.zeros((N_CORES, P, T), dtype=np.float32)
    dinvn[n_core, n_slot, n_t] = dinv
    dinv2n = np.zeros((N_CORES, P, T), dtype=np.float32)
    dinv2n[n_core, n_slot, n_t] = dinv * dinv
    rdinv = np.zeros((N_CORES, 1, NPC), dtype=np.float32)
    rdinv[n_core, 0, n_t * P + n_slot] = rdinv_node

    # --- per-core transposed, tile-blocked node features --------------
    KI = -(-IN // P)
    IN_pad = KI * P
    # xt[c, p_in, (t*KI+ki)*P + n] = x[node(c,t,n), ki*P + p_in]
    xf = np.zeros((N_CORES, T, P, IN_pad), dtype=np.float32)
    xf[n_core, n_t, n_slot, :IN] = x
    xt = (
        xf.reshape(N_CORES, T, P, KI, P)
        .transpose(0, 4, 1, 3, 2)
        .reshape(N_CORES, P, T * KI * P)
    )

    # per-(local tile, window) idx counts, maxed over cores and rounded
    # up to whole 128-chunks (SPMD shares one program across cores);
    # trailing all-empty chunks are skipped by the gather.
    cA = np.bincount(e_tile[toA], minlength=n_tiles).reshape(N_CORES, T)
    cB = np.bincount(e_tile[~toA], minlength=n_tiles).reshape(N_CORES, T)
    nA_tile = tuple(
        int(x) for x in (-(-cA.max(axis=0) // P) * P).clip(min=P)
    )
    nB_tile = tuple(
        int(x) for x in (-(-cB.max(axis=0) // P) * P).clip(min=P)
    )

    meta = dict(
        N=N, IN=IN, IN_pad=IN_pad, T=T, K_A=K_A, K_B=K_B, K=K_tot,
        NPC=NPC, NG=NG, WA=WA, WB_off=WB_off, agch=tuple(agch),
        nA_tile=nA_tile, nB_tile=nB_tile,
        node_core=n_core, node_col=n_t * P + n_slot,
    )
    return xt, idxT, S, dinvn, dinv2n, rdinv, meta


def _assemble(outs, meta, OUT):
    """Gather per-core outputs back to the original node order."""
    N = meta["N"]
    T = meta["T"]
    full = np.empty((N, OUT), dtype=np.float32)
    node_core = meta["node_core"]
    node_col = meta["node_col"]
    for c in range(N_CORES):
        # device out is [P, T*OUT] (partition-major); node (t, p) is row
        # t*P + p of the logical [NPC, OUT]
        arr = outs[c].reshape(P, T, OUT).transpose(1, 0, 2).reshape(-1, OUT)
        m = node_core == c
        full[m] = arr[node_col[m]]
    return full


# -------------------------------------------------------------- device side


def _build_program(T, K_A, K_B, KI, HID, OUT, NPC, NG, WA, WB_off, agch,
                   nA_tile, nB_tile, n_cores):
    import concourse.bacc as bacc
    import concourse.tile as tile
    from concourse import mybir

    f32 = mybir.dt.float32
    bf16 = mybir.dt.bfloat16
    fp8 = mybir.dt.float8e4
    i16 = mybir.dt.int16
    K = K_A + K_B
    KC = K + 1
    KH = HID // P  # 128-chunks of hidden dim
    Relu = mybir.ActivationFunctionType.Relu
    Copy = mybir.ActivationFunctionType.Copy
    BOFF = T * K_A * 8  # idx-table column base of the window-B region
    QMAX = max(t1 - t0 for t0, t1 in agch)
    GBUFS = 5  # gather-tile double-buffer depth

    nc = bacc.Bacc(
        "TRN2", target_bir_lowering=False, debug=False, num_devices=n_cores,
        num_swdge_queues=4,
    )

    xt = nc.dram_tensor("xt", [P, T * KI * P], bf16, kind="ExternalInput").ap()
    w1 = nc.dram_tensor("w1", [P, KI * HID], bf16, kind="ExternalInput").ap()
    b1 = nc.dram_tensor("b1", [1, HID], bf16, kind="ExternalInput").ap()
    w2 = nc.dram_tensor("w2", [P, KH * OUT], bf16, kind="ExternalInput").ap()
    b2 = nc.dram_tensor("b2", [1, OUT], bf16, kind="ExternalInput").ap()
    s_in = nc.dram_tensor("s", [P, T * KC * P], fp8, kind="ExternalInput").ap()
    dinvn = nc.dram_tensor("dinvn", [P, T], f32, kind="ExternalInput").ap()
    dinv2n = nc.dram_tensor("dinv2n", [P, T], f32, kind="ExternalInput").ap()
    rdinv = nc.dram_tensor("rdinv", [1, NPC], bf16, kind="ExternalInput").ap()
    idxt = nc.dram_tensor(
        "idxt", [P, max(T * K * 8, 8)], i16, kind="ExternalInput"
    ).ap()
    out = nc.dram_tensor("out", [P, T * OUT], f32, kind="ExternalOutput").ap()

    rg = [list(range(n_cores))]
    qn = [0]

    def next_q():
        q = qn[0]
        qn[0] = (q + 1) % 4
        return q

    with tile.TileContext(nc) as tc:
        with (
            tc.tile_pool(name="dram", bufs=1, space="DRAM") as dpool,
            tc.tile_pool(name="const", bufs=1) as cpool,
            tc.tile_pool(name="work", bufs=2) as wpool,
            tc.tile_pool(name="gath", bufs=2) as gpool,
            tc.tile_pool(name="sblk", bufs=2) as spool,
            tc.tile_pool(name="pers", bufs=1) as ppool,
            tc.tile_pool(name="ps", bufs=2, space="PSUM") as pspool,
        ):
            h1_locq = [
                dpool.tile([P, (t1 - t0) * HID], bf16, name=f"h1loc{i}")
                for i, (t0, t1) in enumerate(agch)
            ]
            h2_locq = [
                dpool.tile([P, (t1 - t0) * OUT], bf16, name=f"h2loc{i}")
                for i, (t0, t1) in enumerate(agch)
            ]
            # One Shared tensor per AllGather chunk (single collective
            # writer each); chunk q holds global rows
            # [8*P*sum(len<q), ...) and doubles as gather window q.
            h1_fullq = [
                dpool.tile([n_cores * P * (t1 - t0), HID], bf16,
                           addr_space="Shared", name=f"h1full{i}")
                for i, (t0, t1) in enumerate(agch)
            ]
            h2_fullq = [
                dpool.tile([n_cores * P * (t1 - t0), OUT], bf16,
                           addr_space="Shared", name=f"h2full{i}")
                for i, (t0, t1) in enumerate(agch)
            ]

            # ---- constants -----------------------------------------------
            w1_sb = cpool.tile([P, KI * HID], bf16)
            nc.sync.dma_start(out=w1_sb[:], in_=w1[:])
            w2_sb = cpool.tile([P, KH * OUT], bf16)
            nc.sync.dma_start(out=w2_sb[:], in_=w2[:])
            b1_sb = cpool.tile([1, HID], bf16)
            nc.sync.dma_start(out=b1_sb[:], in_=b1[:])
            b2_sb = cpool.tile([1, OUT], bf16)
            nc.sync.dma_start(out=b2_sb[:], in_=b2[:])
            dinvn_sb = cpool.tile([P, T], f32)
            nc.sync.dma_start(out=dinvn_sb[:], in_=dinvn[:])
            dinv2n_sb = cpool.tile([P, T], f32)
            nc.sync.dma_start(out=dinv2n_sb[:], in_=dinv2n[:])
            rdinv_sb = cpool.tile([1, NPC], bf16)
            nc.sync.dma_start(out=rdinv_sb[:], in_=rdinv[:])
            idx_sb = cpool.tile([P, max(T * K * 8, 8)], i16)
            nc.sync.dma_start(out=idx_sb[:], in_=idxt[:])

            a1T = ppool.tile([P, KH * NPC], bf16)  # transposed activations

            for _ in range(GBUFS):
                z1 = gpool.tile([P, BT * max(K_A, 1) * HID], bf16, tag="gA",
                                name="z1", bufs=GBUFS)
                nc.gpsimd.memset(z1[:], 0.0)
                if K_B > 0:
                    z2 = gpool.tile([P, BT * K_B * HID], bf16, tag="gB",
                                    name="z2", bufs=GBUFS)
                    nc.gpsimd.memset(z2[:], 0.0)

            def gathers(t0, bb, h_fullq, h_locq, qi, tq0, F, tag):
                """Batched windowed dma_gathers + self-row DMA for dst
                tiles [t0, t0+bb); returns (ti, k) -> [128, F] slice."""
                gA = gpool.tile([P, BT * max(K_A, 1) * HID], bf16,
                                tag=tag + "A", name="gA", bufs=GBUFS)
                if K_A > 0:
                    for ti in range(bb):
                        t = t0 + ti
                        n_idx = nA_tile[t]
                        nc.gpsimd.dma_gather(
                            out_ap=gA[:, ti * K_A * F:
                                      ti * K_A * F + (n_idx // P) * F
                                      ].rearrange("p (c e) -> p c e", e=F),
                            in_ap=h_fullq[0][0:WA, :],
                            idxs_ap=idx_sb[:, t * K_A * 8:(t + 1) * K_A * 8],
                            num_idxs=n_idx,
                            num_idxs_reg=n_idx,
                            elem_size=F,
                            single_packet=False,
                            queue_num=next_q(),
                        )
                gB = None
                if K_B > 0:
                    gB = gpool.tile([P, BT * K_B * HID], bf16,
                                    tag=tag + "B", name="gB", bufs=GBUFS)
                    for ti in range(bb):
                        t = t0 + ti
                        n_idx = nB_tile[t]
                        nc.gpsimd.dma_gather(
                            out_ap=gB[:, ti * K_B * F:
                                      ti * K_B * F + (n_idx // P) * F
                                      ].rearrange("p (c e) -> p c e", e=F),
                            in_ap=h_fullq[0][WB_off:NG, :],
                            idxs_ap=idx_sb[
                                :, BOFF + t * K_B * 8:BOFF + (t + 1) * K_B * 8
                            ],
                            num_idxs=n_idx,
                            num_idxs_reg=n_idx,
                            elem_size=F,
                            single_packet=False,
                            queue_num=next_q(),
                        )
                gS = gpool.tile([P, BT * HID], bf16, tag=tag + "S", name="gS",
                                bufs=GBUFS)
                nc.sync.dma_start(
                    out=gS[:, :bb * F],
                    in_=h_locq[qi][:, (t0 - tq0) * F:(t0 - tq0 + bb) * F],
                )

                def chunk(ti, k):
                    if k < K_A:
                        c = ti * K_A + k
                        return gA[:, c * F:(c + 1) * F]
                    if k < K:
                        c = ti * K_B + (k - K_A)
                        return gB[:, c * F:(c + 1) * F]
                    return gS[:, ti * F:(ti + 1) * F]

                return chunk

            def load_s(t0, bb):
                s_sb = spool.tile([P, BT * KC * P], fp8, tag="s", name="s_sb",
                                  bufs=3)
                nc.sync.dma_start(
                    out=s_sb[:, :bb * KC * P],
                    in_=s_in[:, t0 * KC * P:(t0 + bb) * KC * P],
                )
                return s_sb

            # ---- layer-1 GEMM: h1 = dinv * (x @ W1), chunked AllGather ---
            for qi, (tq0, tq1) in enumerate(agch):
                qlen = tq1 - tq0
                for t0, bb in _batches(tq0, tq1, BT):
                    xtb = wpool.tile([P, BT * KI * P], bf16, tag="xt",
                                     name="xtb", bufs=3)
                    nc.sync.dma_start(
                        out=xtb[:, :bb * KI * P],
                        in_=xt[:, t0 * KI * P:(t0 + bb) * KI * P],
                    )
                    h1b = wpool.tile([P, BT * HID], bf16, tag="h1b",
                                     name="h1b", bufs=3)
                    for ti in range(bb):
                        t = t0 + ti
                        ps_h = pspool.tile([P, HID], f32, tag="ps_h")
                        for ki in range(KI):
                            nc.tensor.matmul(
                                ps_h[:],
                                lhsT=xtb[:, (ti * KI + ki) * P:
                                         (ti * KI + ki + 1) * P],
                                rhs=w1_sb[:, ki * HID:(ki + 1) * HID],
                                start=(ki == 0),
                                stop=(ki == KI - 1),
                            )
                        nc.vector.tensor_scalar_mul(
                            h1b[:, ti * HID:(ti + 1) * HID],
                            ps_h[:],
                            dinvn_sb[:, t:t + 1],
                        )
                    nc.sync.dma_start(
                        out=h1_locq[qi][
                            :, (t0 - tq0) * HID:(t0 - tq0 + bb) * HID],
                        in_=h1b[:, :bb * HID],
                    )
                nc.gpsimd.collective_compute(
                    "AllGather",
                    mybir.AluOpType.bypass,
                    replica_groups=rg,
                    ins=[h1_locq[qi].opt()],
                    outs=[h1_fullq[qi].opt()],
                )

            # ---- layer-1 aggregation (transposed) + relu + layer-2 GEMM --
            for qi, (tq0, tq1) in enumerate(agch):
                qlen = tq1 - tq0
                for t0, bb in _batches(tq0, tq1, BT):
                    chunk = gathers(t0, bb, h1_fullq, h1_locq, qi, tq0, HID,
                                    "g")
                    s_sb = load_s(t0, bb)
                    h2b = wpool.tile([P, BT * OUT], bf16, tag="h2b",
                                     name="h2b", bufs=3)
                    for ti in range(bb):
                        t = t0 + ti
                        psa_t = pspool.tile([P, KH * P], f32, tag="ps_a",
                                            bufs=3)
                        psa = [psa_t[:, fh * P:(fh + 1) * P]
                               for fh in range(KH)]
                        for fh in range(KH):
                            nc.tensor.matmul(
                                psa[fh][:],
                                lhsT=b1_sb[0:1, fh * P:(fh + 1) * P],
                                rhs=rdinv_sb[0:1, t * P:(t + 1) * P],
                                start=True,
                                stop=False,
                            )
                        for k in range(KC):
                            g = chunk(ti, k)
                            sc = (ti * KC + k) * P
                            for fh in range(KH):
                                nc.tensor.matmul(
                                    psa[fh][:],
                                    lhsT=g[:, fh * P:(fh + 1) * P],
                                    rhs=s_sb[:, sc:sc + P],
                                    start=False,
                                    stop=(k == KC - 1),
                                )
                        for fh in range(KH):
                            nc.scalar.activation(
                                out=a1T[:, fh * NPC + t * P:
                                        fh * NPC + (t + 1) * P],
                                in_=psa[fh][:],
                                func=Relu,
                            )
                        # layer-2 GEMM: h2 = dinv^2 * (a1 @ W2)
                        ps2 = pspool.tile([P, OUT], f32, tag="ps_o")
                        for kh in range(KH):
                            nc.tensor.matmul(
                                ps2[:],
                                lhsT=a1T[:, kh * NPC + t * P:
                                         kh * NPC + (t + 1) * P],
                                rhs=w2_sb[:, kh * OUT:(kh + 1) * OUT],
                                start=(kh == 0),
                                stop=(kh == KH - 1),
                            )
                        nc.vector.tensor_scalar_mul(
                            h2b[:, ti * OUT:(ti + 1) * OUT],
                            ps2[:],
                            dinv2n_sb[:, t:t + 1],
                        )
                    nc.sync.dma_start(
                        out=h2_locq[qi][
                            :, (t0 - tq0) * OUT:(t0 - tq0 + bb) * OUT],
                        in_=h2b[:, :bb * OUT],
                    )
                nc.gpsimd.collective_compute(
                    "AllGather",
                    mybir.AluOpType.bypass,
                    replica_groups=rg,
                    ins=[h2_locq[qi].opt()],
                    outs=[h2_fullq[qi].opt()],
                )

            # ---- layer-2 aggregation: out = dinv * (S^T @ h2[idx]) + b2 --
            for qi, (tq0, tq1) in enumerate(agch):
                for t0, bb in _batches(tq0, tq1, BT):
                    chunk = gathers(t0, bb, h2_fullq, h2_locq, qi, tq0, OUT,
                                    "g")
                    s_sb = load_s(t0, bb)
                    ob = wpool.tile([P, BT * OUT], f32, tag="ob", name="ob")
                    for ti in range(bb):
                        t = t0 + ti
                        pso = pspool.tile([P, OUT], f32, tag="ps_o")
                        nc.tensor.matmul(
                            pso[:],
                            lhsT=rdinv_sb[0:1, t * P:(t + 1) * P],
                            rhs=b2_sb[0:1, :],
                            start=True,
                            stop=False,
                        )
                        for k in range(KC):
                            sc = (ti * KC + k) * P
                            nc.tensor.matmul(
                                pso[:],
                                lhsT=s_sb[:, sc:sc + P],
                                rhs=chunk(ti, k),
                                start=False,
                                stop=(k == KC - 1),
                            )
                        nc.scalar.activation(
                            out=ob[:, ti * OUT:(ti + 1) * OUT],
                            in_=pso[:],
                            func=Copy,
                            scale=dinvn_sb[:, t:t + 1],
                        )
                    nc.sync.dma_start(
                        out=out[:, t0 * OUT:(t0 + bb) * OUT],
                        in_=ob[:, :bb * OUT],
                    )

    nc.compile()
    return nc


def _get_program(T, K_A, K_B, KI, HID, OUT, NPC, NG, WA, WB_off, agch,
                 nA_tile, nB_tile, n_cores=N_CORES):
    key = (T, K_A, K_B, KI, HID, OUT, NPC, NG, WA, WB_off, agch,
           nA_tile, nB_tile, n_cores)
    if key not in _prog_cache:
        _prog_cache[key] = _build_program(
            T, K_A, K_B, KI, HID, OUT, NPC, NG, WA, WB_off, agch,
            nA_tile, nB_tile, n_cores
        )
    return _prog_cache[key]


# ------------------------------------------------------------------- driver


def _make_in_maps(x, edge_index, W1, b1, W2, b2):
    W1 = np.ascontiguousarray(np.asarray(W1, dtype=np.float32))
    W2 = np.ascontiguousarray(np.asarray(W2, dtype=np.float32))
    b1 = np.ascontiguousarray(np.asarray(b1, dtype=np.float32)).reshape(1, -1)
    b2 = np.ascontiguousarray(np.asarray(b2, dtype=np.float32)).reshape(1, -1)
    xt, idxT, S, dinvn, dinv2n, rdinv, meta = _preprocess(x, edge_index)
    IN_pad = meta["IN_pad"]
    KI = IN_pad // P
    HID = W1.shape[1]
    OUT = W2.shape[1]
    if W1.shape[0] < IN_pad:
        W1 = np.concatenate(
            [W1, np.zeros((IN_pad - W1.shape[0], HID), np.float32)], axis=0
        )
    # w1 tiled: [P, KI*HID], block ki = W1[ki*P:(ki+1)*P, :]
    w1t = W1.reshape(KI, P, HID).transpose(1, 0, 2).reshape(P, KI * HID)
    KH = HID // P
    w2t = W2.reshape(KH, P, OUT).transpose(1, 0, 2).reshape(P, KH * OUT)

    in_maps = [
        {
            "xt": _bf16(xt[c]),
            "w1": _bf16(w1t),
            "b1": _bf16(b1),
            "w2": _bf16(w2t),
            "b2": _bf16(b2),
            "s": _fp8(S[c]),
            "dinvn": dinvn[c],
            "dinv2n": dinv2n[c],
            "rdinv": _bf16(rdinv[c]),
            "idxt": idxT[c],
        }
        for c in range(N_CORES)
    ]
    return in_maps, meta, HID, OUT


def run(x, edge_index, W1, b1, W2, b2, trace=False, trace_cores=None):
    from concourse.bass_utils import run_bass_kernel_spmd

    in_maps, meta, HID, OUT = _make_in_maps(x, edge_index, W1, b1, W2, b2)
    nc = _get_program(
        meta["T"], meta["K_A"], meta["K_B"], meta["IN_pad"] // P, HID, OUT,
        meta["NPC"], meta["NG"], meta["WA"], meta["WB_off"], meta["agch"],
        meta["nA_tile"], meta["nB_tile"],
    )
    res = run_bass_kernel_spmd(
        nc,
        in_maps,
        core_ids=list(range(N_CORES)),
        trace=trace,
        trace_cores=trace_cores,
    )
    outs = [res.results[c]["out"] for c in range(N_CORES)]
    return _assemble(outs, meta, OUT), res


def kernel(x, edge_index, W1, b1, W2, b2):
    full, _ = run(x, edge_index, W1, b1, W2, b2, trace=False)
    return full


# revision 24
# speedup vs baseline: 1.0260x; 1.0260x over previous
"""Two-layer GCN (PyG GCNConv-style) on 8 Trainium2 NeuronCores.

Strategy: nodes are partitioned across the 8 cores (load-balanced into
128-row tiles by in-degree), edges partitioned by destination node so
the segment-sum is local.  Each layer is transform-first: local GEMM
(h = x @ W, scaled by dinv[src]), AllGather of the bf16 transformed
features, then a local gather + segment-sum over incoming edges.

Key optimizations over the v1 kernel (1.04 ms):
- S (the one-hot segment-sum matrix) is fp8 {0,1}: the dinv[dst]
  normalization is factored out of S — dinv[src] is folded into the
  AllGathered features (as before), dinv[dst] is applied after the
  aggregation.  For layer 1, relu(dinv*x + b1) = dinv*relu(x + b1*rdinv)
  lets the dinv[dst] scale commute past the relu and fold into the
  layer-2 GEMM output scale (dinv^2 instead of dinv); the bias enters
  through the PSUM-init outer-product matmul with rdinv = 1/dinv.
  This halves the dense-S DMA traffic and keeps everything exact.
- Gathers are batched: one dma_gather per (batch of 4 dst tiles,
  int16 window) instead of per tile, cutting the per-instruction SWDGE
  ucode overhead (~1 us each) 6.5x.  All other DMAs (x-tiles, S tiles,
  self rows, outputs) are batched the same way to cut the ~600 ns
  per-dma_start sequencer cost.
- The AllGathers are chunked (tiles [0,16), [16,32), [32,49)) so the
  collective overlaps the GEMM (layer 1) / aggregation (layer 2) that
  produces its input.  The global feature table is laid out
  chunk-major, then core-major, then partition-major so each chunk is
  a contiguous AllGather destination; the gather indices address this
  layout directly.

The segment-sum runs on the TensorEngine: for each destination tile of
128 nodes, its incoming edges (chunked by 128) are bulk-gathered with
dma_gather into SBUF [128edges x F] per chunk (bf16), and contracted
with the one-hot S.  Layer-1 aggregation is computed TRANSPOSED
(psxT[f,d] = sum_k G_k^T @ S_k) so the relu output lands directly in
lhsT layout for the layer-2 GEMM - no on-device transposes at all.

dma_gather takes int16 row indices, so the gathered table is addressed
through two overlapping <=32767-row windows.
"""

import numpy as np

P = 128
N_CORES = 8
WINDOW_CAP = 32512  # dma_gather int16 window (multiple of 128, <= 32767)
BT = 2  # dst tiles per gather/DMA batch

_prog_cache = {}


# ---------------------------------------------------------------- host side


def _bf16(a):
    import ml_dtypes

    return np.asarray(a, dtype=ml_dtypes.bfloat16)


def _fp8(a):
    import ml_dtypes

    return np.asarray(a, dtype=ml_dtypes.float8_e4m3)


def _ag_chunks(T, B):
    """One AllGather per layer (a Shared tensor allows only a single
    collective writer, and splitting the gather table into per-chunk
    windows inflates the per-tile chunk count ~12%)."""
    return [(0, T)]


def _batches(t0, t1, B):
    return [(s, min(s + B, t1) - s) for s in range(t0, t1, B)]


def _preprocess(x, edge_index):
    """Partition nodes/edges, build per-core device arrays."""
    x = np.ascontiguousarray(np.asarray(x, dtype=np.float32))
    ei = np.asarray(edge_index)
    N, IN = x.shape

    src = ei[0].astype(np.int64)
    dst = ei[1].astype(np.int64)

    deg = 1 + np.bincount(dst, minlength=N)  # with self loop, >= 1
    dinv = (1.0 / np.sqrt(deg.astype(np.float64))).astype(np.float32)
    rdinv_node = np.sqrt(deg.astype(np.float64)).astype(np.float32)

    npc_nodes = -(-N // N_CORES)
    T = -(-npc_nodes // P)  # dst tiles per core
    NPC = T * P  # node slots per core
    n_tiles = N_CORES * T
    NG = n_tiles * P  # global node slots

    # --- pack nodes into tiles, balancing per-tile in-degree (LPT) ----
    import heapq

    degg = deg - 1  # gathered (non-self) in-degree
    tile_of = np.empty(N, dtype=np.int64)
    pos_of = np.empty(N, dtype=np.int64)
    counts = np.zeros(n_tiles, dtype=np.int64)
    loads = np.zeros(n_tiles, dtype=np.int64)
    order = np.argsort(-degg, kind="stable")
    heap = [(0, t) for t in range(n_tiles)]
    heapq.heapify(heap)
    deg_l = degg[order]
    for i in range(N):
        v = order[i]
        while True:
            load, t = heapq.heappop(heap)
            if counts[t] < P:
                break
        tile_of[v] = t
        pos_of[v] = counts[t]
        counts[t] += 1
        load += int(deg_l[i])
        loads[t] = load
        if counts[t] < P:
            heapq.heappush(heap, (load, t))

    # repair pass: move small nodes off overloaded tiles to reach the
    # ideal chunk count ceil(total/(n_tiles*P)) if possible
    K_ideal = max(1, int(-(-int(degg.sum()) // (n_tiles * P))))
    target = K_ideal * P
    if loads.max() > target:
        by_tile = [[] for _ in range(n_tiles)]
        for i in range(N - 1, -1, -1):  # ascending degree order
            by_tile[tile_of[order[i]]].append(order[i])
        free = [(loads[t], t) for t in range(n_tiles)
                if counts[t] < P and loads[t] < target]
        heapq.heapify(free)
        for t_over in np.flatnonzero(loads > target):
            stack = by_tile[t_over]
            si = 0
            while loads[t_over] > target and si < len(stack) and free:
                v = stack[si]
                si += 1
                d = int(degg[v])
                moved = False
                tried = []
                while free:
                    lo, t2 = heapq.heappop(free)
                    if lo != loads[t2] or counts[t2] >= P:
                        continue  # stale
                    if loads[t2] + d <= target:
                        tile_of[v] = t2
                        pos_of[v] = counts[t2]
                        counts[t2] += 1
                        loads[t2] += d
                        loads[t_over] -= d
                        moved = True
                        if counts[t2] < P and loads[t2] < target:
                            heapq.heappush(free, (loads[t2], t2))
                        break
                    tried.append((lo, t2))
                for it in tried:
                    heapq.heappush(free, it)
                if not moved:
                    break
        # recompute pos_of consistently (holes possible after moves)
        ordv = np.lexsort((np.arange(N), tile_of))
        pos = np.empty(N, dtype=np.int64)
        tt = tile_of[ordv]
        st = np.zeros(n_tiles + 1, dtype=np.int64)
        np.cumsum(np.bincount(tt, minlength=n_tiles), out=st[1:])
        pos[ordv] = np.arange(N) - st[tt]
        pos_of = pos

    K = max(1, int(-(-loads.max() // P)))  # min gather chunks per dst tile

    # --- global table row of each node: chunk-major, core-major, then
    # partition-major / tile-minor within the chunk (matches the
    # p-major flattening of the per-core [P, qlen*F] AllGather input).
    agch = _ag_chunks(T, BT)
    chunk_of_t = np.empty(T, dtype=np.int64)
    chunk_t0 = np.empty(T, dtype=np.int64)
    chunk_len = np.empty(T, dtype=np.int64)
    rowbase_t = np.empty(T, dtype=np.int64)
    base = 0
    for qi, (tq0, tq1) in enumerate(agch):
        qlen = tq1 - tq0
        chunk_of_t[tq0:tq1] = qi
        chunk_t0[tq0:tq1] = tq0
        chunk_len[tq0:tq1] = qlen
        rowbase_t[tq0:tq1] = base
        base += N_CORES * P * qlen

    n_core = (tile_of // T).astype(np.int64)
    n_t = tile_of % T
    # row(q, c, p, t) = rowbase_q + c*P*qlen + p*qlen + (t - tq0)
    row_of = (
        rowbase_t[n_t]
        + n_core * P * chunk_len[n_t]
        + pos_of * chunk_len[n_t]
        + (n_t - chunk_t0[n_t])
    )

    # --- per-edge placement (non-self edges) --------------------------
    e_tile = tile_of[dst]
    e_dslot = pos_of[dst].astype(np.int64)
    e_srcrow = row_of[src]

    sort_idx = np.lexsort((e_srcrow, e_tile))
    e_tile = e_tile[sort_idx]
    e_dslot = e_dslot[sort_idx]
    e_srcrow = e_srcrow[sort_idx]
    nE = len(e_tile)

    # --- window split (dma_gather int16 limit) ------------------------
    WA = min(WINDOW_CAP, NG)  # window A = rows [0, WA)
    WB_off = max(NG - WINDOW_CAP, 0)  # window B = rows [WB_off, NG)
    use_B = WB_off > 0

    tile_n = np.bincount(e_tile, minlength=n_tiles)
    if use_B:
        mustA = e_srcrow < WB_off
        mustB = e_srcrow >= WA
        flex = ~mustA & ~mustB
        cntA = np.bincount(e_tile[mustA], minlength=n_tiles)
        cntB = np.bincount(e_tile[mustB], minlength=n_tiles)
        # find (K_A, K_B) with K_A+K_B minimal and all tiles feasible
        found = None
        K_tot = K
        while found is None:
            mid = -(-K_tot // 2)
            for d in range(K_tot + 1):
                for K_A in {mid + d, mid - d}:
                    if not 0 <= K_A <= K_tot:
                        continue
                    K_B = K_tot - K_A
                    if (
                        cntA.max() <= K_A * P
                        and cntB.max() <= K_B * P
                        and tile_n.max() <= (K_A + K_B) * P
                    ):
                        found = (K_A, K_B)
                        break
                if found:
                    break
            if not found:
                K_tot += 1
        K_A, K_B = found
        capB = K_B * P
        # how many of each tile's flex edges go to window A
        nA_t = np.minimum(K_A * P, cntA + np.bincount(
            e_tile[flex], minlength=n_tiles))
        nA_t = np.maximum(nA_t, tile_n - capB)
        flexA_quota = nA_t - cntA
        flex_idx = np.flatnonzero(flex)
        ft = e_tile[flex_idx]
        fstart = np.zeros(n_tiles + 1, dtype=np.int64)
        np.cumsum(np.bincount(ft, minlength=n_tiles), out=fstart[1:])
        frank = np.arange(len(ft)) - fstart[ft]
        toA = mustA.copy()
        toA[flex_idx[frank < flexA_quota[ft]]] = True
    else:
        K_A, K_B = K, 0
        toA = np.ones(nE, dtype=bool)
    K_tot = K_A + K_B
    KC = K_tot + 1  # chunk columns per tile incl. the self chunk

    # --- chunk/slot assignment within each (tile, window) -------------
    e_j = np.empty(nE, dtype=np.int64)  # slot within its window list
    e_idxval = np.empty(nE, dtype=np.int64)  # int16 index value
    for is_A in (True, False):
        m = toA if is_A else ~toA
        if not m.any():
            continue
        idxs = np.flatnonzero(m)
        t_sel = e_tile[idxs]
        start = np.zeros(n_tiles + 1, dtype=np.int64)
        np.cumsum(np.bincount(t_sel, minlength=n_tiles), out=start[1:])
        e_j[idxs] = np.arange(len(idxs)) - start[t_sel]
        e_idxval[idxs] = e_srcrow[idxs] - (0 if is_A else WB_off)

    e_kloc = e_j // P  # chunk within window
    e_p = e_j % P
    e_chunk = np.where(toA, e_kloc, K_A + e_kloc)  # chunk within tile

    e_core = e_tile // T
    e_t_in_core = e_tile % T

    # idx table: window-A blocks first (tile t at cols [t*K_A*8, ...)),
    # then window-B blocks at T*K_A*8 + t*K_B*8.  Value for position j
    # of a tile's window list sits at [j%16, base + j//16], replicated
    # across the 8 groups of 16 partitions.
    idx_cols = T * K_tot * 8
    idx16 = np.zeros((N_CORES, 16, max(idx_cols, 8)), dtype=np.int16)
    blk_base = np.where(
        toA,
        e_t_in_core * K_A * 8,
        T * K_A * 8 + e_t_in_core * K_B * 8,
    )
    idx16[e_core, e_j % 16, blk_base + e_j // 16] = e_idxval.astype(np.int16)
    idxT = np.tile(idx16, (1, P // 16, 1))

    # compact S descriptor: dslot[p, t*KC + k] = destination slot of the
    # edge at (chunk k, partition p) of tile t, or 255 (no edge).  The
    # device expands it to the one-hot fp8 S block with a single
    # is_equal against an iota row (vector engine, otherwise idle).
    dslot = np.full((N_CORES, P, T * KC), 255.0, dtype=np.float32)
    col = e_t_in_core * KC + e_chunk
    dslot[e_core, e_p, col] = e_dslot

    # self chunk (k == K_tot): identity
    n_slot = pos_of
    dslot[n_core, n_slot, n_t * KC + K_tot] = n_slot
    iota = np.tile(np.arange(P, dtype=np.float32), (P, 1))

    # per-node scale vectors; 0 for empty slots
    dinvn = np.zeros((N_CORES, P, T), dtype=np.float32)
    dinvn[n_core, n_slot, n_t] = dinv
    dinv2n = np.zeros((N_CORES, P, T), dtype=np.float32)
    dinv2n[n_core, n_slot, n_t] = dinv * dinv
    rdinv = np.zeros((N_CORES, 1, NPC), dtype=np.float32)
    rdinv[n_core, 0, n_t * P + n_slot] = rdinv_node

    # --- per-core transposed, tile-blocked node features --------------
    KI = -(-IN // P)
    IN_pad = KI * P
    # xt[c, p_in, (t*KI+ki)*P + n] = x[node(c,t,n), ki*P + p_in]
    xf = np.zeros((N_CORES, T, P, IN_pad), dtype=np.float32)
    xf[n_core, n_t, n_slot, :IN] = x
    xt = (
        xf.reshape(N_CORES, T, P, KI, P)
        .transpose(0, 4, 1, 3, 2)
        .reshape(N_CORES, P, T * KI * P)
    )

    # per-(local tile, window) idx counts, maxed over cores and rounded
    # up to whole 128-chunks (SPMD shares one program across cores);
    # trailing all-empty chunks are skipped by the gather.
    cA = np.bincount(e_tile[toA], minlength=n_tiles).reshape(N_CORES, T)
    cB = np.bincount(e_tile[~toA], minlength=n_tiles).reshape(N_CORES, T)
    nA_tile = tuple(
        int(x) for x in (-(-cA.max(axis=0) // P) * P).clip(min=P)
    )
    nB_tile = tuple(
        int(x) for x in (-(-cB.max(axis=0) // P) * P).clip(min=P)
    )

    meta = dict(
        N=N, IN=IN, IN_pad=IN_pad, T=T, K_A=K_A, K_B=K_B, K=K_tot,
        NPC=NPC, NG=NG, WA=WA, WB_off=WB_off, agch=tuple(agch),
        nA_tile=nA_tile, nB_tile=nB_tile,
        node_core=n_core, node_col=n_t * P + n_slot,
    )
    return xt, idxT, dslot, iota, dinvn, dinv2n, rdinv, meta


def _assemble(outs, meta, OUT):
    """Gather per-core outputs back to the original node order."""
    N = meta["N"]
    T = meta["T"]
    full = np.empty((N, OUT), dtype=np.float32)
    node_core = meta["node_core"]
    node_col = meta["node_col"]
    for c in range(N_CORES):
        # device out is [P, T*OUT] (partition-major); node (t, p) is row
        # t*P + p of the logical [NPC, OUT]
        arr = outs[c].reshape(P, T, OUT).transpose(1, 0, 2).reshape(-1, OUT)
        m = node_core == c
        full[m] = arr[node_col[m]]
    return full


# -------------------------------------------------------------- device side


def _build_program(T, K_A, K_B, KI, HID, OUT, NPC, NG, WA, WB_off, agch,
                   nA_tile, nB_tile, n_cores):
    import concourse.bacc as bacc
    import concourse.bass as bass
    import concourse.tile as tile
    from concourse import mybir

    f32 = mybir.dt.float32
    bf16 = mybir.dt.bfloat16
    fp8 = mybir.dt.float8e4
    i16 = mybir.dt.int16
    K = K_A + K_B
    KC = K + 1
    KH = HID // P  # 128-chunks of hidden dim
    Relu = mybir.ActivationFunctionType.Relu
    Copy = mybir.ActivationFunctionType.Copy
    BOFF = T * K_A * 8  # idx-table column base of the window-B region
    QMAX = max(t1 - t0 for t0, t1 in agch)
    GBUFS = 5  # gather-tile double-buffer depth

    nc = bacc.Bacc(
        "TRN2", target_bir_lowering=False, debug=False, num_devices=n_cores,
        num_swdge_queues=4,
    )

    xt = nc.dram_tensor("xt", [P, T * KI * P], bf16, kind="ExternalInput").ap()
    w1 = nc.dram_tensor("w1", [P, KI * HID], bf16, kind="ExternalInput").ap()
    b1 = nc.dram_tensor("b1", [1, HID], bf16, kind="ExternalInput").ap()
    w2 = nc.dram_tensor("w2", [P, KH * OUT], bf16, kind="ExternalInput").ap()
    b2 = nc.dram_tensor("b2", [1, OUT], bf16, kind="ExternalInput").ap()
    dslot_in = nc.dram_tensor(
        "dslot", [P, T * KC], bf16, kind="ExternalInput"
    ).ap()
    iota_in = nc.dram_tensor("iota", [P, P], bf16, kind="ExternalInput").ap()
    dinvn = nc.dram_tensor("dinvn", [P, T], f32, kind="ExternalInput").ap()
    dinv2n = nc.dram_tensor("dinv2n", [P, T], f32, kind="ExternalInput").ap()
    rdinv = nc.dram_tensor("rdinv", [1, NPC], bf16, kind="ExternalInput").ap()
    idxt = nc.dram_tensor(
        "idxt", [P, max(T * K * 8, 8)], i16, kind="ExternalInput"
    ).ap()
    out = nc.dram_tensor("out", [P, T * OUT], f32, kind="ExternalOutput").ap()

    rg = [list(range(n_cores))]
    qn = [0]

    def next_q():
        q = qn[0]
        qn[0] = (q + 1) % 4
        return q

    with tile.TileContext(nc) as tc:
        with (
            tc.tile_pool(name="dram", bufs=1, space="DRAM") as dpool,
            tc.tile_pool(name="const", bufs=1) as cpool,
            tc.tile_pool(name="work", bufs=2) as wpool,
            tc.tile_pool(name="gath", bufs=2) as gpool,
            tc.tile_pool(name="sblk", bufs=2) as spool,
            tc.tile_pool(name="pers", bufs=1) as ppool,
            tc.tile_pool(name="ps", bufs=2, space="PSUM") as pspool,
        ):
            h1_locq = [
                dpool.tile([P, (t1 - t0) * HID], bf16, name=f"h1loc{i}")
                for i, (t0, t1) in enumerate(agch)
            ]
            h2_locq = [
                dpool.tile([P, (t1 - t0) * OUT], bf16, name=f"h2loc{i}")
                for i, (t0, t1) in enumerate(agch)
            ]
            # One Shared tensor per AllGather chunk (single collective
            # writer each); chunk q holds global rows
            # [8*P*sum(len<q), ...) and doubles as gather window q.
            h1_fullq = [
                dpool.tile([n_cores * P * (t1 - t0), HID], bf16,
                           addr_space="Shared", name=f"h1full{i}")
                for i, (t0, t1) in enumerate(agch)
            ]
            h2_fullq = [
                dpool.tile([n_cores * P * (t1 - t0), OUT], bf16,
                           addr_space="Shared", name=f"h2full{i}")
                for i, (t0, t1) in enumerate(agch)
            ]

            # ---- constants -----------------------------------------------
            w1_sb = cpool.tile([P, KI * HID], bf16)
            nc.sync.dma_start(out=w1_sb[:], in_=w1[:])
            w2_sb = cpool.tile([P, KH * OUT], bf16)
            nc.sync.dma_start(out=w2_sb[:], in_=w2[:])
            b1_sb = cpool.tile([1, HID], bf16)
            nc.sync.dma_start(out=b1_sb[:], in_=b1[:])
            b2_sb = cpool.tile([1, OUT], bf16)
            nc.sync.dma_start(out=b2_sb[:], in_=b2[:])
            dinvn_sb = cpool.tile([P, T], f32)
            nc.sync.dma_start(out=dinvn_sb[:], in_=dinvn[:])
            dinv2n_sb = cpool.tile([P, T], f32)
            nc.sync.dma_start(out=dinv2n_sb[:], in_=dinv2n[:])
            rdinv_sb = cpool.tile([1, NPC], bf16)
            nc.sync.dma_start(out=rdinv_sb[:], in_=rdinv[:])
            idx_sb = cpool.tile([P, max(T * K * 8, 8)], i16)
            nc.sync.dma_start(out=idx_sb[:], in_=idxt[:])
            dslot_sb = cpool.tile([P, T * KC], bf16)
            nc.sync.dma_start(out=dslot_sb[:], in_=dslot_in[:])
            iota_sb = cpool.tile([P, P], bf16)
            nc.sync.dma_start(out=iota_sb[:], in_=iota_in[:])

            a1T = ppool.tile([P, KH * NPC], bf16)  # transposed activations

            for _ in range(GBUFS):
                z1 = gpool.tile([P, BT * max(K_A, 1) * HID], bf16, tag="gA",
                                name="z1", bufs=GBUFS)
                nc.gpsimd.memset(z1[:], 0.0)
                if K_B > 0:
                    z2 = gpool.tile([P, BT * K_B * HID], bf16, tag="gB",
                                    name="z2", bufs=GBUFS)
                    nc.gpsimd.memset(z2[:], 0.0)

            def gathers(t0, bb, h_fullq, h_locq, qi, tq0, F, tag):
                """Batched windowed dma_gathers + self-row DMA for dst
                tiles [t0, t0+bb); returns (ti, k) -> [128, F] slice."""
                gA = gpool.tile([P, BT * max(K_A, 1) * HID], bf16,
                                tag=tag + "A", name="gA", bufs=GBUFS)
                if K_A > 0:
                    for ti in range(bb):
                        t = t0 + ti
                        n_idx = nA_tile[t]
                        nc.gpsimd.dma_gather(
                            out_ap=gA[:, ti * K_A * F:
                                      ti * K_A * F + (n_idx // P) * F
                                      ].rearrange("p (c e) -> p c e", e=F),
                            in_ap=h_fullq[0][0:WA, :],
                            idxs_ap=idx_sb[:, t * K_A * 8:(t + 1) * K_A * 8],
                            num_idxs=n_idx,
                            num_idxs_reg=n_idx,
                            elem_size=F,
                            single_packet=False,
                            queue_num=next_q(),
                        )
                gB = None
                if K_B > 0:
                    gB = gpool.tile([P, BT * K_B * HID], bf16,
                                    tag=tag + "B", name="gB", bufs=GBUFS)
                    for ti in range(bb):
                        t = t0 + ti
                        n_idx = nB_tile[t]
                        nc.gpsimd.dma_gather(
                            out_ap=gB[:, ti * K_B * F:
                                      ti * K_B * F + (n_idx // P) * F
                                      ].rearrange("p (c e) -> p c e", e=F),
                            in_ap=h_fullq[0][WB_off:NG, :],
                            idxs_ap=idx_sb[
                                :, BOFF + t * K_B * 8:BOFF + (t + 1) * K_B * 8
                            ],
                            num_idxs=n_idx,
                            num_idxs_reg=n_idx,
                            elem_size=F,
                            single_packet=False,
                            queue_num=next_q(),
                        )
                gS = gpool.tile([P, BT * HID], bf16, tag=tag + "S", name="gS",
                                bufs=GBUFS)
                nc.sync.dma_start(
                    out=gS[:, :bb * F],
                    in_=h_locq[qi][:, (t0 - tq0) * F:(t0 - tq0 + bb) * F],
                )

                def chunk(ti, k):
                    if k < K_A:
                        c = ti * K_A + k
                        return gA[:, c * F:(c + 1) * F]
                    if k < K:
                        c = ti * K_B + (k - K_A)
                        return gB[:, c * F:(c + 1) * F]
                    return gS[:, ti * F:(ti + 1) * F]

                return chunk

            def load_s(t0, bb):
                # build the one-hot S block on the vector engine:
                # S[p, c*P + d] = (dslot[p, c] == d)
                s_sb = spool.tile([P, BT * KC * P], fp8, tag="s", name="s_sb")
                out = s_sb[:, :bb * KC * P].rearrange("p (c d) -> p c d", d=P)
                v0 = dslot_sb[:, t0 * KC:(t0 + bb) * KC]
                in0 = bass.AP(v0.tensor, v0.offset,
                              [list(v0.ap[0]), list(v0.ap[1]), [0, P]])
                v1 = iota_sb[:]
                in1 = bass.AP(v1.tensor, v1.offset,
                              [list(v1.ap[0]), [0, bb * KC], list(v1.ap[1])])
                nc.vector.tensor_tensor(
                    out, in0, in1, mybir.AluOpType.is_equal
                )
                return s_sb

            # ---- layer-1 GEMM: h1 = dinv * (x @ W1), chunked AllGather ---
            for qi, (tq0, tq1) in enumerate(agch):
                qlen = tq1 - tq0
                for t0, bb in _batches(tq0, tq1, BT):
                    xtb = wpool.tile([P, BT * KI * P], bf16, tag="xt",
                                     name="xtb")
                    nc.sync.dma_start(
                        out=xtb[:, :bb * KI * P],
                        in_=xt[:, t0 * KI * P:(t0 + bb) * KI * P],
                    )
                    h1b = wpool.tile([P, BT * HID], bf16, tag="h1b",
                                     name="h1b", bufs=3)
                    for ti in range(bb):
                        t = t0 + ti
                        ps_h = pspool.tile([P, HID], f32, tag="ps_h")
                        for ki in range(KI):
                            nc.tensor.matmul(
                                ps_h[:],
                                lhsT=xtb[:, (ti * KI + ki) * P:
                                         (ti * KI + ki + 1) * P],
                                rhs=w1_sb[:, ki * HID:(ki + 1) * HID],
                                start=(ki == 0),
                                stop=(ki == KI - 1),
                            )
                        nc.vector.tensor_scalar_mul(
                            h1b[:, ti * HID:(ti + 1) * HID],
                            ps_h[:],
                            dinvn_sb[:, t:t + 1],
                        )
                    nc.sync.dma_start(
                        out=h1_locq[qi][
                            :, (t0 - tq0) * HID:(t0 - tq0 + bb) * HID],
                        in_=h1b[:, :bb * HID],
                    )
                nc.gpsimd.collective_compute(
                    "AllGather",
                    mybir.AluOpType.bypass,
                    replica_groups=rg,
                    ins=[h1_locq[qi].opt()],
                    outs=[h1_fullq[qi].opt()],
                )

            # ---- layer-1 aggregation (transposed) + relu + layer-2 GEMM --
            for qi, (tq0, tq1) in enumerate(agch):
                qlen = tq1 - tq0
                for t0, bb in _batches(tq0, tq1, BT):
                    chunk = gathers(t0, bb, h1_fullq, h1_locq, qi, tq0, HID,
                                    "g")
                    s_sb = load_s(t0, bb)
                    h2b = wpool.tile([P, BT * OUT], bf16, tag="h2b",
                                     name="h2b", bufs=3)
                    for ti in range(bb):
                        t = t0 + ti
                        psa_t = pspool.tile([P, KH * P], f32, tag="ps_a",
                                            bufs=3)
                        psa = [psa_t[:, fh * P:(fh + 1) * P]
                               for fh in range(KH)]
                        for fh in range(KH):
                            nc.tensor.matmul(
                                psa[fh][:],
                                lhsT=b1_sb[0:1, fh * P:(fh + 1) * P],
                                rhs=rdinv_sb[0:1, t * P:(t + 1) * P],
                                start=True,
                                stop=False,
                            )
                        for k in range(KC):
                            g = chunk(ti, k)
                            sc = (ti * KC + k) * P
                            for fh in range(KH):
                                nc.tensor.matmul(
                                    psa[fh][:],
                                    lhsT=g[:, fh * P:(fh + 1) * P],
                                    rhs=s_sb[:, sc:sc + P],
                                    start=False,
                                    stop=(k == KC - 1),
                                )
                        for fh in range(KH):
                            nc.scalar.activation(
                                out=a1T[:, fh * NPC + t * P:
                                        fh * NPC + (t + 1) * P],
                                in_=psa[fh][:],
                                func=Relu,
                            )
                        # layer-2 GEMM: h2 = dinv^2 * (a1 @ W2)
                        ps2 = pspool.tile([P, OUT], f32, tag="ps_o")
                        for kh in range(KH):
                            nc.tensor.matmul(
                                ps2[:],
                                lhsT=a1T[:, kh * NPC + t * P:
                                         kh * NPC + (t + 1) * P],
                                rhs=w2_sb[:, kh * OUT:(kh + 1) * OUT],
                                start=(kh == 0),
                                stop=(kh == KH - 1),
                            )
                        nc.vector.tensor_scalar_mul(
                            h2b[:, ti * OUT:(ti + 1) * OUT],
                            ps2[:],
                            dinv2n_sb[:, t:t + 1],
                        )
                    nc.sync.dma_start(
                        out=h2_locq[qi][
                            :, (t0 - tq0) * OUT:(t0 - tq0 + bb) * OUT],
                        in_=h2b[:, :bb * OUT],
                    )
                nc.gpsimd.collective_compute(
                    "AllGather",
                    mybir.AluOpType.bypass,
                    replica_groups=rg,
                    ins=[h2_locq[qi].opt()],
                    outs=[h2_fullq[qi].opt()],
                )

            # ---- layer-2 aggregation: out = dinv * (S^T @ h2[idx]) + b2 --
            for qi, (tq0, tq1) in enumerate(agch):
                for t0, bb in _batches(tq0, tq1, BT):
                    chunk = gathers(t0, bb, h2_fullq, h2_locq, qi, tq0, OUT,
                                    "g")
                    s_sb = load_s(t0, bb)
                    ob = wpool.tile([P, BT * OUT], f32, tag="ob", name="ob")
                    for ti in range(bb):
                        t = t0 + ti
                        pso = pspool.tile([P, OUT], f32, tag="ps_o")
                        nc.tensor.matmul(
                            pso[:],
                            lhsT=rdinv_sb[0:1, t * P:(t + 1) * P],
                            rhs=b2_sb[0:1, :],
                            start=True,
                            stop=False,
                        )
                        for k in range(KC):
                            sc = (ti * KC + k) * P
                            nc.tensor.matmul(
                                pso[:],
                                lhsT=s_sb[:, sc:sc + P],
                                rhs=chunk(ti, k),
                                start=False,
                                stop=(k == KC - 1),
                            )
                        nc.scalar.activation(
                            out=ob[:, ti * OUT:(ti + 1) * OUT],
                            in_=pso[:],
                            func=Copy,
                            scale=dinvn_sb[:, t:t + 1],
                        )
                    nc.sync.dma_start(
                        out=out[:, t0 * OUT:(t0 + bb) * OUT],
                        in_=ob[:, :bb * OUT],
                    )

    nc.compile()
    return nc


def _get_program(T, K_A, K_B, KI, HID, OUT, NPC, NG, WA, WB_off, agch,
                 nA_tile, nB_tile, n_cores=N_CORES):
    key = (T, K_A, K_B, KI, HID, OUT, NPC, NG, WA, WB_off, agch,
           nA_tile, nB_tile, n_cores)
    if key not in _prog_cache:
        _prog_cache[key] = _build_program(
            T, K_A, K_B, KI, HID, OUT, NPC, NG, WA, WB_off, agch,
            nA_tile, nB_tile, n_cores
        )
    return _prog_cache[key]


# ------------------------------------------------------------------- driver


def _make_in_maps(x, edge_index, W1, b1, W2, b2):
    W1 = np.ascontiguousarray(np.asarray(W1, dtype=np.float32))
    W2 = np.ascontiguousarray(np.asarray(W2, dtype=np.float32))
    b1 = np.ascontiguousarray(np.asarray(b1, dtype=np.float32)).reshape(1, -1)
    b2 = np.ascontiguousarray(np.asarray(b2, dtype=np.float32)).reshape(1, -1)
    xt, idxT, dslot, iota, dinvn, dinv2n, rdinv, meta = _preprocess(
        x, edge_index)
    IN_pad = meta["IN_pad"]
    KI = IN_pad // P
    HID = W1.shape[1]
    OUT = W2.shape[1]
    if W1.shape[0] < IN_pad:
        W1 = np.concatenate(
            [W1, np.zeros((IN_pad - W1.shape[0], HID), np.float32)], axis=0
        )
    # w1 tiled: [P, KI*HID], block ki = W1[ki*P:(ki+1)*P, :]
    w1t = W1.reshape(KI, P, HID).transpose(1, 0, 2).reshape(P, KI * HID)
    KH = HID // P
    w2t = W2.reshape(KH, P, OUT).transpose(1, 0, 2).reshape(P, KH * OUT)

    in_maps = [
        {
            "xt": _bf16(xt[c]),
            "w1": _bf16(w1t),
            "b1": _bf16(b1),
            "w2": _bf16(w2t),
            "b2": _bf16(b2),
            "dslot": _bf16(dslot[c]),
            "iota": _bf16(iota),
            "dinvn": dinvn[c],
            "dinv2n": dinv2n[c],
            "rdinv": _bf16(rdinv[c]),
            "idxt": idxT[c],
        }
        for c in range(N_CORES)
    ]
    return in_maps, meta, HID, OUT


def run(x, edge_index, W1, b1, W2, b2, trace=False, trace_cores=None):
    from concourse.bass_utils import run_bass_kernel_spmd

    in_maps, meta, HID, OUT = _make_in_maps(x, edge_index, W1, b1, W2, b2)
    nc = _get_program(
        meta["T"], meta["K_A"], meta["K_B"], meta["IN_pad"] // P, HID, OUT,
        meta["NPC"], meta["NG"], meta["WA"], meta["WB_off"], meta["agch"],
        meta["nA_tile"], meta["nB_tile"],
    )
    res = run_bass_kernel_spmd(
        nc,
        in_maps,
        core_ids=list(range(N_CORES)),
        trace=trace,
        trace_cores=trace_cores,
    )
    outs = [res.results[c]["out"] for c in range(N_CORES)]
    return _assemble(outs, meta, OUT), res


def kernel(x, edge_index, W1, b1, W2, b2):
    full, _ = run(x, edge_index, W1, b1, W2, b2, trace=False)
    return full
